# revision 1
# baseline (speedup 1.0000x reference)
"""Trainium2 Bass kernel: AttentionPooling (attention-weighted global_add_pool).

Computes, for x [N, 256], sorted graph ids batch [N] (num_graphs=4096):
    h    = tanh(x @ W1 + b1)            # [N, 128]
    attn = h @ W2 + b2                  # [N, 1]
    out  = segment_sum(x * attn, batch) # [4096, 256]

Strategy: data-parallel over nodes on 8 NeuronCores. Per core, nodes are
processed in 128-row tiles. Per tile on-device:
  - PE transpose x tile halves -> xT (d on partitions)
  - hT[a, n] = sum_d W1[d, a] * xT[d, n]  (two K=128 matmuls, PSUM acc)
  - th = tanh(hT + b1) on ScalarE (bias is per-partition since partitions = a)
  - attn[n, 1] = th.T @ W2 (matmul, lhsT = th)
  - S[n, j] = (rel[n] == j) * (attn[n] + b2)   (one fused DVE tensor_scalar;
    rel[n] = batch[n] - first_graph_of_window, precomputed on host)
  - acc[j, d] += S.T @ x_tile  (PSUM accumulation across a window of tiles;
    the host guarantees every window spans < 32 distinct graphs)
Window accumulators [32, 256] are flushed raw to DRAM; the host maps window
slot j -> graph g0[w] + j and sums across windows/cores (cheap: ~8 MB).
"""

import math

import numpy as np

import concourse.bass as bass
import concourse.mybir as mybir
import concourse.tile as tile
from concourse import bacc, bass_utils

P = 128
D_IN = 256
D_ATT = 128
G_WIN = 32  # one-hot width = max graphs a window may span

N_NODES = 500_000
NUM_GRAPHS = 4096
N_CORES = 8
NODES_PER_CORE = N_NODES // N_CORES  # 62500
TILES_PER_CORE = math.ceil(NODES_PER_CORE / P)  # 489
NPC_PAD = TILES_PER_CORE * P  # 62592

F32 = mybir.dt.float32
F32R = mybir.dt.float32r


def build_program(n_tiles: int, win_tiles: int, b2: float,
                  mm_f32r: bool = False, tr_f32r: bool = False,
                  proc_tiles: int | None = None):
    """Build the single-core Bass program (same NEFF runs SPMD on all cores).

    proc_tiles < n_tiles processes only a prefix of the tiles (same input
    shapes) — used to measure device time differentially through the
    high-overhead axon tunnel."""
    assert n_tiles % win_tiles == 0, "pad tiles to a whole number of windows"
    if proc_tiles is None:
        proc_tiles = n_tiles
    assert proc_tiles % win_tiles == 0
    n_wins = proc_tiles // win_tiles
    nc = bacc.Bacc(trn_type="TRN2", target_bir_lowering=False, debug=False,
                   num_devices=N_CORES)

    # all constants packed into one tensor -> one DMA -> one wait at the
    # first consumer (HW limits sync-wait slots per instruction)
    n_const = 2 * D_ATT + 1 + 1 + P + G_WIN + n_tiles
    # x is host-swizzled to [n_wins, 128, win_tiles*256] so each window's
    # DMA is partition-contiguous (16 KB/partition, 128 descriptors) —
    # a partition-strided view of row-major x was descriptor-bound (~1 GB/s).
    x_d = nc.dram_tensor("x", [(n_tiles // win_tiles) * P, win_tiles * D_IN],
                         F32, kind="ExternalInput").ap()
    cst_d = nc.dram_tensor("cst", [P, n_const], F32, kind="ExternalInput").ap()
    out_d = nc.dram_tensor("out", [n_wins * G_WIN, D_IN], F32,
                           kind="ExternalOutput").ap()

    def r(ap):
        return ap.bitcast(F32R) if mm_f32r else ap

    def rt(ap):
        return ap.bitcast(F32R) if tr_f32r else ap

    with tile.TileContext(nc) as tc:
        with (
            tc.tile_pool(name="consts", bufs=1) as cpool,
            tc.tile_pool(name="xin", bufs=3) as xpool,
            tc.tile_pool(name="xtsb", bufs=3) as xtpool,
            tc.tile_pool(name="thsb", bufs=3) as thpool,
            tc.tile_pool(name="attnsb", bufs=3) as apool,
            tc.tile_pool(name="ssb", bufs=4) as spool,
            tc.tile_pool(name="outsb", bufs=2) as opool,
            tc.tile_pool(name="xtps", bufs=2, space="PSUM") as xtps_pool,
            tc.tile_pool(name="htps", bufs=2, space="PSUM") as htps_pool,
            tc.tile_pool(name="atps", bufs=2, space="PSUM") as atps_pool,
            tc.tile_pool(name="accps", bufs=2, space="PSUM") as accps_pool,
        ):
            cst_sb = cpool.tile([P, n_const], F32, name="cst_sb")
            nc.sync.dma_start(out=cst_sb, in_=cst_d)
            o = 0
            w1_sb = cst_sb[:, o:o + 2 * D_ATT]; o += 2 * D_ATT
            b1_sb = cst_sb[:, o:o + 1]; o += 1
            w2_sb = cst_sb[:, o:o + 1]; o += 1
            idn_sb = cst_sb[:, o:o + P]; o += P
            iota_sb = cst_sb[:, o:o + G_WIN]; o += G_WIN
            relT_sb = cst_sb[:, o:o + n_tiles]; o += n_tiles

            for w in range(n_wins):
                t0 = w * win_tiles
                wt = win_tiles

                x_chunk = xpool.tile([P, wt * D_IN], F32, name="x_chunk",
                                     tag="x_chunk")
                nc.sync.dma_start(
                    out=x_chunk, in_=x_d[w * P:(w + 1) * P, :])

                acc_ps = accps_pool.tile([G_WIN, D_IN], F32, name="acc_ps",
                                         tag="acc_ps")

                groups = [tuple(range(g, min(g + 2, wt)))
                          for g in range(0, wt, 2)]
                for gi, grp in enumerate(groups):
                    ng = len(grp)
                    # --- transposes: xT for each tile in the group ---
                    xt_ps = xtps_pool.tile([P, ng * D_IN], F32, name="xt_ps",
                                           tag="xt_ps")
                    for i, lt in enumerate(grp):
                        x_tile = x_chunk[:, lt * D_IN:(lt + 1) * D_IN]
                        nc.tensor.transpose(
                            rt(xt_ps[:, i * D_IN:i * D_IN + P]),
                            rt(x_tile[:, 0:P]), rt(idn_sb))
                        nc.tensor.transpose(
                            rt(xt_ps[:, i * D_IN + P:(i + 1) * D_IN]),
                            rt(x_tile[:, P:D_IN]), rt(idn_sb))
                    # PSUM -> SBUF copy. One engine per group (alternating
                    # DVE/ACT for balance) so each xt_ps buffer has a single
                    # reader engine: matmuls may carry at most 2 sync waits,
                    # so every PE instruction must depend on <= 2 engines.
                    xt_sb = xtpool.tile([P, ng * D_IN], F32, name="xt_sb",
                                        tag="xt_sb")
                    if gi % 2 == 0:
                        nc.vector.tensor_copy(xt_sb, xt_ps[:, 0:ng * D_IN])
                    else:
                        nc.scalar.copy(xt_sb, xt_ps[:, 0:ng * D_IN])

                    # --- hT = W1h.T @ xT accumulated over the two d-halves ---
                    ht_ps = htps_pool.tile([P, ng * D_ATT], F32, name="ht_ps",
                                           tag="ht_ps")
                    xt4 = xt_sb.rearrange("p (t h n) -> p t h n", t=ng, h=2)
                    ht3 = ht_ps.rearrange("p (t n) -> p t n", t=ng)
                    nc.tensor.matmul(ht3, r(w1_sb[:, 0:D_ATT]),
                                     r(xt4[:, :, 0, :]), start=True, stop=False)
                    nc.tensor.matmul(ht3, r(w1_sb[:, D_ATT:2 * D_ATT]),
                                     r(xt4[:, :, 1, :]), start=False, stop=True)

                    # --- th = tanh(hT + b1) ---
                    th_sb = thpool.tile([P, ng * D_ATT], F32, name="th_sb",
                                        tag="th_sb")
                    nc.scalar.activation(th_sb, ht_ps[:, 0:ng * D_ATT],
                                         mybir.ActivationFunctionType.Tanh,
                                         bias=b1_sb, scale=1.0)

                    # --- attn[n] = th.T @ W2 ---
                    at_ps = atps_pool.tile([P, ng], F32, name="at_ps",
                                           tag="at_ps")
                    for i in range(ng):
                        nc.tensor.matmul(at_ps[:, i:i + 1],
                                         r(th_sb[:, i * D_ATT:(i + 1) * D_ATT]),
                                         r(w2_sb), start=True, stop=True)
                    at_sb = apool.tile([P, ng], F32, name="at_sb", tag="at_sb")
                    nc.vector.tensor_scalar_add(at_sb, at_ps[:, 0:ng],
                                                float(b2))

                    # --- S = (iota == rel) * attn' ; acc += S.T @ x ---
                    for i, lt in enumerate(grp):
                        gt = t0 + lt
                        s_sb = spool.tile([P, G_WIN], F32, name="s_sb",
                                          tag="s_sb")
                        nc.vector.tensor_scalar(
                            s_sb, iota_sb, relT_sb[:, gt:gt + 1],
                            at_sb[:, i:i + 1],
                            mybir.AluOpType.is_equal, mybir.AluOpType.mult)
                        x_tile = x_chunk[:, lt * D_IN:(lt + 1) * D_IN]
                        nc.tensor.matmul(acc_ps, r(s_sb), r(x_tile),
                                         start=(lt == 0), stop=(lt == wt - 1))

                # --- flush window accumulator (DVE: shares the wait lane
                # with the S-build so the next window's first mS matmul
                # stays within the 2-sync-wait matmul limit) ---
                out_sb = opool.tile([G_WIN, D_IN], F32, name="out_sb",
                                    tag="out_sb")
                nc.vector.tensor_copy(out_sb, acc_ps)
                nc.sync.dma_start(
                    out=out_d[w * G_WIN:(w + 1) * G_WIN, :], in_=out_sb)

    nc.compile()
    return nc


F16 = mybir.dt.float16


def build_program_f16c(n_tiles: int, win_tiles: int, b2: float,
                       proc_tiles: int | None = None):
    """fp16-compensated variant: x and W1 are split on the host into fp16
    hi + lo planes (x = x_h + x_l exactly to ~2^-22 rel). All large matmuls
    run in fp16 (1 cyc/row vs fp32's 4) keeping 3 of the 4 cross terms, so
    the result carries ~2^-21 relative error instead of fp32's ~2^-24:
      hT  = W1h.T@xTh + W1h.T@xTl + W1l.T@xTh      (per d-half)
      out = Sh.T@xh + Sh.T@xl + Sl.T@xh
    where Sh/Sl are the one-hot selection matrices scaled by the fp16
    hi/lo split of attn (exact products: one-hot entries are 0/1).
    The attn dot itself (th.T @ W2) stays fp32: its lhsT free size is 1,
    so fp32's stream penalty is irrelevant there."""
    assert n_tiles % win_tiles == 0
    if proc_tiles is None:
        proc_tiles = n_tiles
    assert proc_tiles % win_tiles == 0
    n_wins = proc_tiles // win_tiles
    nc = bacc.Bacc(trn_type="TRN2", target_bir_lowering=False, debug=False,
                   num_devices=N_CORES)

    n_const = 1 + 1 + G_WIN + n_tiles                 # b1 | w2 | iota | relT
    n_const16 = 4 * D_ATT + P                         # W1 hi/lo halves | idn
    # x16: per window [128, win_tiles*512] fp16; per tile 512 cols =
    # 256 hi || 256 lo (host-swizzled, partition-contiguous)
    x_d = nc.dram_tensor("x16", [(n_tiles // win_tiles) * P, win_tiles * 512],
                         F16, kind="ExternalInput").ap()
    cst_d = nc.dram_tensor("cst", [P, n_const], F32, kind="ExternalInput").ap()
    c16_d = nc.dram_tensor("cst16", [P, n_const16], F16,
                           kind="ExternalInput").ap()
    out_d = nc.dram_tensor("out", [n_wins * G_WIN, D_IN], F32,
                           kind="ExternalOutput").ap()

    TW = 512  # fp16 cols per tile in the x chunk

    with tile.TileContext(nc) as tc:
        with (
            tc.tile_pool(name="consts", bufs=1) as cpool,
            tc.tile_pool(name="xin", bufs=3) as xpool,
            tc.tile_pool(name="xtsb", bufs=3) as xtpool,
            tc.tile_pool(name="thsb", bufs=3) as thpool,
            tc.tile_pool(name="attnsb", bufs=3) as apool,
            tc.tile_pool(name="ssb", bufs=4) as spool,
            tc.tile_pool(name="outsb", bufs=2) as opool,
            tc.tile_pool(name="xtps", bufs=2, space="PSUM") as xtps_pool,
            tc.tile_pool(name="htps", bufs=2, space="PSUM") as htps_pool,
            tc.tile_pool(name="atps", bufs=2, space="PSUM") as atps_pool,
            tc.tile_pool(name="accps", bufs=2, space="PSUM") as accps_pool,
        ):
            cst_sb = cpool.tile([P, n_const], F32, name="cst_sb")
            nc.sync.dma_start(out=cst_sb, in_=cst_d)
            o = 0
            b1_sb = cst_sb[:, o:o + 1]; o += 1
            w2_sb = cst_sb[:, o:o + 1]; o += 1
            iota_sb = cst_sb[:, o:o + G_WIN]; o += G_WIN
            relT_sb = cst_sb[:, o:o + n_tiles]; o += n_tiles

            c16_sb = cpool.tile([P, n_const16], F16, name="c16_sb")
            nc.sync.dma_start(out=c16_sb, in_=c16_d)
            w1h = [c16_sb[:, 0:P], c16_sb[:, P:2 * P]]          # fp16(W1)
            w1l = [c16_sb[:, 2 * P:3 * P], c16_sb[:, 3 * P:4 * P]]
            idn_sb = c16_sb[:, 4 * P:5 * P]

            for w in range(n_wins):
                t0 = w * win_tiles
                wt = win_tiles

                x_chunk = xpool.tile([P, wt * TW], F16, name="x_chunk",
                                     tag="x_chunk")
                nc.sync.dma_start(out=x_chunk, in_=x_d[w * P:(w + 1) * P, :])

                acc_ps = accps_pool.tile([G_WIN, D_IN], F32, name="acc_ps",
                                         tag="acc_ps")

                groups = [tuple(range(g, min(g + 2, wt)))
                          for g in range(0, wt, 2)]
                for gi, grp in enumerate(groups):
                    ng = len(grp)
                    # --- 4 transposes per tile: (hi|lo) x (d-half 0|1) ---
                    xt_ps = xtps_pool.tile([P, ng * TW], F16, name="xt_ps",
                                           tag="xt_ps")
                    for i, lt in enumerate(grp):
                        for q in range(4):  # hi0, hi1, lo0, lo1
                            nc.tensor.transpose(
                                xt_ps[:, i * TW + q * P:i * TW + (q + 1) * P],
                                x_chunk[:, lt * TW + q * P:
                                        lt * TW + (q + 1) * P],
                                idn_sb)
                    xt_sb = xtpool.tile([P, ng * TW], F16, name="xt_sb",
                                        tag="xt_sb")
                    if gi % 2 == 0:
                        nc.vector.tensor_copy(xt_sb, xt_ps[:, 0:ng * TW])
                    else:
                        nc.scalar.copy(xt_sb, xt_ps[:, 0:ng * TW])

                    # --- hT: 3 fp16 terms per d-half, f32 PSUM accumulate ---
                    ht_ps = htps_pool.tile([P, ng * D_ATT], F32, name="ht_ps",
                                           tag="ht_ps")
                    xt4 = xt_sb.rearrange("p (t q n) -> p t q n", t=ng, q=4)
                    ht3 = ht_ps.rearrange("p (t n) -> p t n", t=ng)
                    terms = [(w1h[0], 0), (w1h[1], 1),      # W1h . xh
                             (w1l[0], 0), (w1l[1], 1),      # W1l . xh
                             (w1h[0], 2), (w1h[1], 3)]      # W1h . xl
                    for k, (wsl, q) in enumerate(terms):
                        nc.tensor.matmul(ht3, wsl, xt4[:, :, q, :],
                                         start=(k == 0),
                                         stop=(k == len(terms) - 1))

                    # --- th = tanh(hT + b1), fp32 ---
                    th_sb = thpool.tile([P, ng * D_ATT], F32, name="th_sb",
                                        tag="th_sb")
                    nc.scalar.activation(th_sb, ht_ps[:, 0:ng * D_ATT],
                                         mybir.ActivationFunctionType.Tanh,
                                         bias=b1_sb, scale=1.0)

                    # --- attn = th.T @ W2 (fp32, free dim 1) ---
                    at_ps = atps_pool.tile([P, ng], F32, name="at_ps",
                                           tag="at_ps")
                    for i in range(ng):
                        nc.tensor.matmul(at_ps[:, i:i + 1],
                                         th_sb[:, i * D_ATT:(i + 1) * D_ATT],
                                         w2_sb, start=True, stop=True)

                    # --- attn' = attn + b2 split into fp16 hi + lo ---
                    ah16 = apool.tile([P, ng], F16, name="ah16", tag="ah16")
                    nc.vector.tensor_scalar_add(ah16, at_ps[:, 0:ng],
                                                float(b2))
                    ah32 = apool.tile([P, ng], F32, name="ah32", tag="ah32")
                    nc.vector.tensor_copy(ah32, ah16)
                    al32 = apool.tile([P, ng], F32, name="al32", tag="al32")
                    for i in range(ng):
                        nc.vector.tensor_scalar(
                            al32[:, i:i + 1], at_ps[:, i:i + 1], float(b2),
                            ah32[:, i:i + 1],
                            mybir.AluOpType.add, mybir.AluOpType.subtract)

                    # --- Sh/Sl one-hots; 3 fp16 pooling terms ---
                    for i, lt in enumerate(grp):
                        gt = t0 + lt
                        sh = spool.tile([P, G_WIN], F16, name="sh", tag="sh")
                        nc.vector.tensor_scalar(
                            sh, iota_sb, relT_sb[:, gt:gt + 1],
                            ah32[:, i:i + 1],
                            mybir.AluOpType.is_equal, mybir.AluOpType.mult)
                        sl = spool.tile([P, G_WIN], F16, name="sl", tag="sl")
                        nc.vector.tensor_scalar(
                            sl, iota_sb, relT_sb[:, gt:gt + 1],
                            al32[:, i:i + 1],
                            mybir.AluOpType.is_equal, mybir.AluOpType.mult)
                        xh_tile = x_chunk[:, lt * TW:lt * TW + D_IN]
                        xl_tile = x_chunk[:, lt * TW + D_IN:(lt + 1) * TW]
                        first = (lt == 0)
                        last = (lt == wt - 1)
                        nc.tensor.matmul(acc_ps, sh, xh_tile,
                                         start=first, stop=False)
                        nc.tensor.matmul(acc_ps, sh, xl_tile,
                                         start=False, stop=False)
                        nc.tensor.matmul(acc_ps, sl, xh_tile,
                                         start=False, stop=last)

                out_sb = opool.tile([G_WIN, D_IN], F32, name="out_sb",
                                    tag="out_sb")
                nc.vector.tensor_copy(out_sb, acc_ps)
                nc.sync.dma_start(
                    out=out_d[w * G_WIN:(w + 1) * G_WIN, :], in_=out_sb)

    nc.compile()
    return nc


def prep_core_f16c(x_real, batch_real, n_tiles, win_tiles):
    """Like prep_core but packs x as interleaved fp16 hi/lo planes:
    per tile 512 cols = 256 hi || 256 lo, window-swizzled."""
    assert n_tiles % win_tiles == 0
    npad = n_tiles * P
    n_real = x_real.shape[0]
    x_pad = np.zeros((npad, D_IN), dtype=np.float32)
    x_pad[:n_real] = x_real
    x_h = x_pad.astype(np.float16)
    x_l = (x_pad - x_h.astype(np.float32)).astype(np.float16)
    xx = np.concatenate([x_h, x_l], axis=1)  # [npad, 512]
    n_wins = n_tiles // win_tiles
    x_sw = np.ascontiguousarray(
        xx.reshape(n_wins, win_tiles, P, 512).transpose(0, 2, 1, 3)
    ).reshape(n_wins * P, win_tiles * 512)

    b = np.full(npad, -1, dtype=np.int64)
    b[:n_real] = batch_real
    rel = np.full(npad, -1.0, dtype=np.float32)
    g0s = np.zeros(n_wins, dtype=np.int64)
    for w in range(n_wins):
        s = w * win_tiles * P
        e = (w + 1) * win_tiles * P
        seg = b[s:e]
        realm = seg >= 0
        g0 = int(seg[realm][0]) if realm.any() else 0
        g0s[w] = g0
        rw = (seg - g0).astype(np.float32)
        rw[~realm] = -1.0
        assert rw.max() < G_WIN
        rel[s:e] = rw
    relT = np.ascontiguousarray(rel.reshape(n_tiles, P).T)
    return x_sw, relT, g0s


def make_consts_f16c(W1, b1, W2):
    """Returns (cst_f32 [128, 34], cst16 [128, 640])."""
    W1 = np.asarray(W1, dtype=np.float32)
    cst = np.ascontiguousarray(np.concatenate([
        np.asarray(b1, np.float32).reshape(P, 1),
        np.asarray(W2, np.float32).reshape(P, 1),
        np.broadcast_to(np.arange(G_WIN, dtype=np.float32), (P, G_WIN)),
    ], axis=1))
    w1h = W1.astype(np.float16)
    w1lf = W1 - w1h.astype(np.float32)
    w1l = w1lf.astype(np.float16)
    cst16 = np.ascontiguousarray(np.concatenate([
        w1h[0:P, :], w1h[P:2 * P, :], w1l[0:P, :], w1l[P:2 * P, :],
        np.eye(P, dtype=np.float16),
    ], axis=1))
    return cst, cst16


def choose_win_tiles(batch_slices, n_tiles):
    """Pick the biggest window size (in tiles) such that every window of
    every core spans < G_WIN distinct graphs (batch is sorted, so the span
    is last - first + 1)."""
    for wt in (16, 8, 4, 2, 1):
        ok = True
        for bc in batch_slices:
            nn = len(bc)
            for s in range(0, nn, wt * P):
                e = min(nn, s + wt * P)
                if bc[e - 1] - bc[s] + 1 > G_WIN - 1:
                    ok = False
                    break
            if not ok:
                break
        if ok:
            return wt
    return 1


def prep_core(x_real, batch_real, n_tiles, win_tiles):
    """Pad one core's slice to n_tiles*128 nodes (whole windows), swizzle x
    per window to a partition-contiguous layout, and build relT + g0s.

    Returns (x_sw [n_wins*128, win_tiles*256] f32, relT [128, n_tiles] f32,
    g0s). Padded nodes get rel = -1 so they never match the one-hot iota.
    x_sw[w*128 + p, t*256:(t+1)*256] = x[(w*win_tiles + t)*128 + p].
    """
    assert n_tiles % win_tiles == 0
    npad = n_tiles * P
    n_real = x_real.shape[0]
    assert n_real <= npad
    x_pad = np.zeros((npad, D_IN), dtype=np.float32)
    x_pad[:n_real] = x_real
    b = np.full(npad, -1, dtype=np.int64)
    b[:n_real] = batch_real

    n_wins = n_tiles // win_tiles
    # [w, t, p, d] -> [w, p, t, d]: window-level partition-major swizzle
    x_sw = np.ascontiguousarray(
        x_pad.reshape(n_wins, win_tiles, P, D_IN).transpose(0, 2, 1, 3)
    ).reshape(n_wins * P, win_tiles * D_IN)

    rel = np.full(npad, -1.0, dtype=np.float32)
    g0s = np.zeros(n_wins, dtype=np.int64)
    for w in range(n_wins):
        s = w * win_tiles * P
        e = (w + 1) * win_tiles * P
        seg = b[s:e]
        realm = seg >= 0
        if realm.any():
            g0 = int(seg[realm][0])  # sorted -> min graph id in window
        else:
            g0 = 0
        g0s[w] = g0
        rw = (seg - g0).astype(np.float32)
        rw[~realm] = -1.0
        assert rw.max() < G_WIN, (
            f"window spans too many graphs: {rw.max()} >= {G_WIN}")
        rel[s:e] = rw
    relT = np.ascontiguousarray(rel.reshape(n_tiles, P).T)
    return x_sw, relT, g0s


def make_consts(W1, b1, W2):
    """Packed constant block [128, 418]: W1-halves | b1 | W2 | I | iota."""
    W1 = np.asarray(W1, dtype=np.float32)
    parts = [
        W1[0:P, :],                                   # [128, 128] = W1 half 0
        W1[P:2 * P, :],                               # [128, 128] = W1 half 1
        np.asarray(b1, np.float32).reshape(P, 1),
        np.asarray(W2, np.float32).reshape(P, 1),
        np.eye(P, dtype=np.float32),
        np.broadcast_to(np.arange(G_WIN, dtype=np.float32), (P, G_WIN)),
    ]
    return np.ascontiguousarray(np.concatenate(parts, axis=1))


def postprocess(raws, g0s_per_core, num_graphs):
    """raws: per-core [n_wins*G_WIN, D_IN] raw window sums -> [G, D_IN]."""
    out = np.zeros((num_graphs, D_IN), dtype=np.float64)
    for raw, g0s in zip(raws, g0s_per_core):
        raw3 = raw.reshape(-1, G_WIN, D_IN)
        for w, g0 in enumerate(g0s):
            width = min(G_WIN, num_graphs - int(g0))
            out[g0:g0 + width] += raw3[w, :width]
    return out.astype(np.float32)


def prepare(x, batch, num_graphs, W1, b1, W2, b2, mode="f16c"):
    """Host-side prep: shard, window metadata, and the Bass program.

    Returns (nc, in_maps, g0s_per_core, num_graphs).
    """
    x = np.asarray(x, dtype=np.float32)
    batch = np.asarray(batch).astype(np.int64)
    num_graphs = int(num_graphs)
    W1 = np.asarray(W1, dtype=np.float32)
    b1 = np.asarray(b1, dtype=np.float32)
    W2 = np.asarray(W2, dtype=np.float32)
    b2f = float(np.asarray(b2).reshape(-1)[0])

    n = x.shape[0]
    assert n == N_NODES and x.shape[1] == D_IN
    assert np.all(np.diff(batch) >= 0), "batch must be sorted"

    # split nodes across cores
    bounds = [(c * NODES_PER_CORE,
               min(n, (c + 1) * NODES_PER_CORE) if c < N_CORES - 1 else n)
              for c in range(N_CORES)]

    wt = choose_win_tiles([batch[s:e] for s, e in bounds], TILES_PER_CORE)
    n_tiles_pad = math.ceil(TILES_PER_CORE / wt) * wt

    in_maps = []
    g0s_per_core = []
    if mode == "f16c":
        cbase, cst16 = make_consts_f16c(W1, b1, W2)
        for s, e in bounds:
            x_sw, relT, g0s = prep_core_f16c(x[s:e], batch[s:e],
                                             n_tiles_pad, wt)
            cst = np.ascontiguousarray(np.concatenate([cbase, relT], axis=1))
            in_maps.append({"x16": x_sw, "cst": cst, "cst16": cst16})
            g0s_per_core.append(g0s)
        nc = build_program_f16c(n_tiles_pad, wt, b2f)
    else:
        cbase = make_consts(W1, b1, W2)
        for s, e in bounds:
            x_sw, relT, g0s = prep_core(x[s:e], batch[s:e], n_tiles_pad, wt)
            cst = np.ascontiguousarray(np.concatenate([cbase, relT], axis=1))
            in_maps.append({"x": x_sw, "cst": cst})
            g0s_per_core.append(g0s)
        nc = build_program(n_tiles_pad, wt, b2f)
    return nc, in_maps, g0s_per_core, num_graphs


def kernel(x, batch, num_graphs, W1, b1, W2, b2):
    nc, in_maps, g0s_per_core, num_graphs = prepare(
        x, batch, num_graphs, W1, b1, W2, b2)
    res = bass_utils.run_bass_kernel_spmd(
        nc, in_maps, core_ids=list(range(N_CORES)))
    raws = [r["out"] for r in res.results]
    return postprocess(raws, g0s_per_core, num_graphs)

